# revision 64
# baseline (speedup 1.0000x reference)
"""AlgebraicTransformerLM on 8 trn2 NeuronCores (Bass/Tile), v2.

Sharding: DP=2 over batch x TP=4 over heads / d_ffn / vocab (cores 0-3 =
batch 0, 4-7 = batch 1). Key points vs the v1 baseline:
  - fp16 weights / activations / attention probabilities everywhere except
    the ALiBi-augmented score matmuls (f32r: the iota rows need f32) and
    the f32 residual stream. fp16 matmul/DVE rates equal bf16 but carry
    8x finer mantissa, so this HALVES the error while halving DMA bytes.
  - Each sublayer AllReduce is split into two 512-token chunks, fired as
    soon as that strip's output projection lands and consumed as late as
    possible (the deferred strip-1 add+norm is emitted between the next
    sublayer's strips), so collectives overlap compute.
  - The residual add rides the collective-output DMA (SWDGE accum_op).
  - Rational softmax: s from PSUM, r = 1/(1+|s|) computed either as
    exp(-ln(1+|s|)) on the ACT LUT or add+reciprocal_approx_fast on DVE
    (per-tile mix balances the engines); w2 = Square(u+1) on ACT;
    causal masking by multiplying a precomputed triangular mask tile;
    w4 = w2*w2 on DVE. Diagonal score tiles are column-sliced so fully
    dead columns are never computed.
  - All activations are pinned to one ACT table set (no reload thrash);
    tiny dependent ldweights heartbeats keep the PE clock un-throttled.
Host prep: fold norm weights/SCALE/0.5 into adjacent matmul weights,
precompute xn0 = norm(emb[ids]) so layer 0 starts without a device norm.
"""
import contextlib
import math

import numpy as np

import concourse.bacc as bacc
import concourse.mybir as mybir
import concourse.tile as tile
from concourse.bass_utils import run_bass_kernel_spmd

F32 = mybir.dt.float32
F32R = mybir.dt.float32r
FP16 = mybir.dt.float16

B, T, V, D, H, L = 2, 1024, 32000, 1024, 16, 4
DFF = 2730
DH = D // H
SCALE = 1.0 / math.sqrt(DH)
EPS = 1e-6

NCORES = 8
TP = 4
HPC = H // TP               # heads per core (4)
FSH = 2 * DH * HPC          # q+k rows per core (512)
VSH = DH * HPC              # v rows per core (256)
DFF_SH = 768                # padded DFF shard (4*768 >= 2730)
NFT_FF = DFF_SH // 128      # 6
VOC_SH = V // TP            # vocab shard per core (8000)
DT = D // 128               # 8
NSTRIP = T // 512           # 2
RG = [[0, 1, 2, 3], [4, 5, 6, 7]]
ALIBI = [2.0 ** (-8.0 * (i + 1) / H) for i in range(H)]

_CACHE = {}

AF = mybir.ActivationFunctionType
ALU = mybir.AluOpType


class PoolSet:
    """Route scratch tags to per-bufs pools."""

    def __init__(self, **pools):
        self._map = pools

    _n = 0

    def tile(self, shape, dtype, tag):
        PoolSet._n += 1
        return self._map[tag].tile(shape, dtype, tag=tag, name=f"{tag}_{PoolSet._n}")


def _causal_tk(s):
    return list(range((s + 1) * (512 // 128)))


def _mask_base(tk, s):
    """affine_select base for tile (tk, strip s): keep where f + base - p >= 0,
    i.e. tq >= tk. None if the whole tile is causal-valid."""
    base = s * 512 - tk * 128
    return base if tk * 128 + 127 > s * 512 else None


def _pin_act_table(arch):
    """Make every activation resolve to natural_log_exp_and_others so the
    ACT table is loaded once instead of ping-ponging between sets (each
    switch costs ~1.3us). Mutates the cached table dict in place; set ids
    keep their original indices so the walrus-side mapping is unchanged."""
    from concourse.hw_specs import get_activation_tables

    tabs = get_activation_tables(arch)
    keep = "natural_log_exp_and_others"
    mine = {AF.Abs, AF.Copy, AF.Square, AF.Exp, AF.Ln}
    if keep not in tabs or not (mine <= tabs[keep]):
        return
    for name, funcs in tabs.items():
        if name != keep:
            funcs -= mine


def build_nc(use_divide=True, use_accum_dma=True):
    nc = bacc.Bacc("TRN2", target_bir_lowering=False)
    _pin_act_table(nc.m.arch)

    x0T = nc.dram_tensor("x0T", [D, T], F32, kind="ExternalInput")
    xn0T = nc.dram_tensor("xn0T", [D, T], FP16, kind="ExternalInput")
    qaug = nc.dram_tensor("qaug", [HPC, 2, T], F32, kind="ExternalInput")
    kaug = nc.dram_tensor("kaug", [HPC, 2, T], F32, kind="ExternalInput")
    wqkT = nc.dram_tensor("wqkT", [L, D, FSH], FP16, kind="ExternalInput")
    wvT = nc.dram_tensor("wvT", [L, D, VSH], FP16, kind="ExternalInput")
    woT = nc.dram_tensor("woT", [L, VSH, D], FP16, kind="ExternalInput")
    wmT = nc.dram_tensor("wmT", [L, D, 2 * DFF_SH], FP16, kind="ExternalInput")
    w3T = nc.dram_tensor("w3T", [L, DFF_SH, D], FP16, kind="ExternalInput")
    membT = nc.dram_tensor("membT", [D, VOC_SH], FP16, kind="ExternalInput")
    logits = nc.dram_tensor("logits", [T, VOC_SH], F32, kind="ExternalOutput")
    NCH = 2 * L * NSTRIP
    cc_in = [nc.dram_tensor(f"cc_in{i}", [D, 512], FP16) for i in range(NCH)]
    cc_out = [nc.dram_tensor(f"cc_out{i}", [D, 512], FP16) for i in range(NCH)]

    with tile.TileContext(nc) as tc, contextlib.ExitStack() as ctx:
        persist = ctx.enter_context(tc.tile_pool(name="persist", bufs=1))
        psA = ctx.enter_context(tc.tile_pool(name="psA", bufs=4, space="PSUM"))
        psAcc = ctx.enter_context(tc.tile_pool(name="psAcc", bufs=2, space="PSUM"))
        psSm = ctx.enter_context(tc.tile_pool(name="psSm", bufs=1, space="PSUM"))

        x = persist.tile([128, DT, T], F32, tag="x")
        nc.sync.dma_start(x[:], x0T[:].rearrange("(dt p) t -> p dt t", p=128))
        xn = persist.tile([128, DT, T], FP16, tag="xn")
        nc.sync.dma_start(xn[:], xn0T[:].rearrange("(dt p) t -> p dt t", p=128))

        ocf = persist.tile([128, 1], F32, tag="ones_colf")
        nc.vector.memset(ocf[:], 1.0)
        ones_colb = persist.tile([128, 1], FP16, tag="ones_colb")
        nc.vector.tensor_copy(ones_colb[:], ocf[:])
        orf = persist.tile([1, 128], F32, tag="ones_rowf")
        nc.vector.memset(orf[:], 1.0)
        ones_rowb = persist.tile([1, 128], FP16, tag="ones_rowb")
        nc.vector.tensor_copy(ones_rowb[:], orf[:])
        ones_b = persist.tile([128, 1], F32, tag="ones_bias")
        nc.vector.memset(ones_b[:], 1.0)

        # master causal mask for diagonal tiles: after column-slicing every
        # diagonal tile keeps elements where f >= p; mask[p, f] = (f >= p)
        cmask = persist.tile([128, 512], FP16, tag="cmask")
        nc.vector.memset(cmask[:], 1.0)
        nc.gpsimd.affine_select(cmask[:], cmask[:], pattern=[[1, 512]],
                                base=0, channel_multiplier=-1,
                                compare_op=ALU.is_ge, fill=0.0)

        def sigrecip(spool, src_ps, n, on_act):
            """r = 1/(1+|s|) from a [128, n] f32 PSUM tile. ACT variant:
            exp(-ln(1+|s|)) via LUT; DVE variant: reciprocal_approx_fast."""
            if on_act:
                a = spool.tile([128, 512], FP16, tag="aa")
                nc.scalar.activation(a[:, :n], src_ps[:, :n], AF.Abs,
                                     scale=1.0)
                ln = spool.tile([128, 512], FP16, tag="rr")
                nc.scalar.activation(ln[:, :n], a[:, :n], AF.Ln, bias=1.0,
                                     scale=1.0)
                r = spool.tile([128, 512], FP16, tag="rb")
                nc.scalar.activation(r[:, :n], ln[:, :n], AF.Exp, scale=-1.0)
            else:
                a = spool.tile([128, 512], F32, tag="af")
                nc.scalar.activation(a[:, :n], src_ps[:, :n], AF.Abs,
                                     scale=1.0)
                d = spool.tile([128, 512], F32, tag="df")
                nc.vector.tensor_scalar(d[:, :n], a[:, :n], scalar1=1.0,
                                        scalar2=None, op0=ALU.add,
                                        op1=ALU.bypass)
                r = spool.tile([128, 512], F32, tag="rf")
                nc.vector.reciprocal_approx_fast(r[:, :n], d[:, :n])
            return r

        def sigpipe(spool, s_ps, w4_out, col0, masked, on_act):
            """w4 = (1 + s/(1+|s|))^4 in fp16 from score psum, columns
            [col0:512] of the original tile live (s_ps holds them in
            [0:512-col0]). masked marks diagonal tiles (causal mask applied
            via the precomputed cmask tile)."""
            n = 512 - col0
            r = sigrecip(spool, s_ps, n, on_act)
            u = spool.tile([128, 512], FP16, tag="uu")
            nc.vector.tensor_tensor(u[:, :n], s_ps[:, :n], r[:, :n],
                                    ALU.mult)
            # HAM heartbeat: a tiny dependent weight load keeps the PE
            # activity monitor from re-throttling during elementwise chains;
            # the next real matmul reloads its own weights regardless.
            nc.tensor.ldweights(u[:, 0:1])
            w2 = spool.tile([128, 512], FP16, tag="w2")
            nc.scalar.activation(w2[:, :n], u[:, :n], AF.Square,
                                 bias=ones_b[:], scale=1.0)
            if masked:
                w2m = spool.tile([128, 512], FP16, tag="w2m")
                nc.gpsimd.tensor_mul(w2m[:, :n], w2[:, :n], cmask[:, :n])
                nc.gpsimd.tensor_mul(w4_out[:, :n], w2m[:, :n], w2m[:, :n])
            else:
                nc.gpsimd.tensor_mul(w4_out[:, :n], w2[:, :n], w2[:, :n])

        def addnorm(s, idx, spool, dpool):
            """x[:, :, strip] += AR-chunk idx; xn strip = normed x (bf16)."""
            sl = slice(s * 512, (s + 1) * 512)
            if use_accum_dma:
                nc.gpsimd.dma_start(
                    x[:, :, sl],
                    cc_out[idx][:].rearrange("(dt p) t -> p dt t", p=128),
                    accum_op=ALU.add)
            else:
                db = dpool.tile([128, DT, 512], FP16, tag="db")
                nc.sync.dma_start(db[:],
                                  cc_out[idx][:].rearrange(
                                      "(dt p) t -> p dt t", p=128))
                for dt in range(DT):
                    eng = nc.vector if dt < 4 else nc.gpsimd
                    eng.tensor_tensor(x[:, dt, sl], x[:, dt, sl], db[:, dt],
                                      ALU.add)
            mag = psSm.tile([1, 512], F32, tag="small", name=f"mag{idx}")
            for half in range(2):
                xa = spool.tile([128, 4, 512], FP16, tag="nabs")
                nc.scalar.activation(xa[:], x[:, 4 * half:4 * half + 4, sl],
                                     AF.Abs, scale=1.0)
                for i in range(4):
                    dt = 4 * half + i
                    nc.tensor.matmul(mag[:], ones_colb[:], xa[:, i],
                                     start=(dt == 0), stop=(dt == DT - 1),
                                     skip_group_check=True)
            md = spool.tile([1, 512], F32, tag="row")
            nc.vector.tensor_scalar(md[:], mag[:], scalar1=1.0 / D,
                                    scalar2=EPS, op0=ALU.mult, op1=ALU.add)
            mr = spool.tile([1, 512], F32, tag="row")
            nc.vector.reciprocal_approx_fast(mr[:], md[:])
            mrb = spool.tile([1, 512], FP16, tag="rowb")
            nc.vector.tensor_copy(mrb[:], mr[:])
            rep = psA.tile([128, 512], F32, tag="ps", name=f"rep{idx}")
            nc.tensor.matmul(rep[:], ones_rowb[:], mrb[:], start=True,
                             stop=True)
            for dt in range(DT):
                nc.vector.tensor_tensor(xn[:, dt, sl], x[:, dt, sl], rep[:],
                                        ALU.mult)

        def attention(l, wpool, apool, dpool, spool, pending):
            # whole-layer weight loads (bf16)
            wqk = wpool.tile([128, DT, FSH], FP16, tag="wqk")
            nc.scalar.dma_start(wqk[:],
                              wqkT[l].rearrange("(dt p) f -> p dt f", p=128))
            wv = wpool.tile([128, DT, VSH], FP16, tag="wv")
            nc.scalar.dma_start(wv[:],
                              wvT[l].rearrange("(dt p) f -> p dt f", p=128))
            wo = wpool.tile([128, 2, D], FP16, tag="wo")
            nc.scalar.dma_start(wo[:],
                              woT[l].rearrange("(pp p) f -> p pp f", p=128))

            vaug = apool.tile([128, DT, HPC * 65], FP16, tag="vaug",
                              name=f"vaug{l}")
            qa = [apool.tile([66, T], F32R, tag=f"qa{h}", name=f"qa{h}_{l}")
                  for h in range(HPC)]
            ka = [apool.tile([66, T], F32R, tag=f"ka{h}", name=f"ka{h}_{l}")
                  for h in range(HPC)]
            for h in range(HPC):
                nc.sync.dma_start(qa[h][64:66, :], qaug[h].bitcast(F32R))
                nc.sync.dma_start(ka[h][64:66, :], kaug[h].bitcast(F32R))
            asb = apool.tile([128, 2, T], FP16, tag="asb", name=f"asb{l}")

            def vproj(s):
                for tt in range(s * 4, s * 4 + 4):
                    ps = psA.tile([128, 512], F32, tag="ps",
                                  name=f"vps{l}_{tt}")
                    for dt in range(DT):
                        nc.tensor.matmul(ps[:, 0:VSH],
                                         xn[:, dt, tt * 128:(tt + 1) * 128],
                                         wv[:, dt], start=(dt == 0),
                                         stop=(dt == DT - 1))
                    for h in range(HPC):
                        nc.vector.tensor_copy(
                            vaug[:, tt, h * 65:h * 65 + 64],
                            ps[:, h * 64:(h + 1) * 64])
                        nc.vector.memset(vaug[:, tt, h * 65 + 64:h * 65 + 65],
                                         1.0)

            def qkproj(s):
                sl = slice(s * 512, (s + 1) * 512)
                for ft in range(4):
                    qk, pair = ft // 2, ft % 2
                    ps = psA.tile([128, 512], F32, tag="ps",
                                  name=f"qkps{l}_{ft}_{s}")
                    for dt in range(DT):
                        nc.tensor.matmul(ps[:],
                                         wqk[:, dt, ft * 128:(ft + 1) * 128],
                                         xn[:, dt, sl], start=(dt == 0),
                                         stop=(dt == DT - 1))
                    tgt = qa if qk == 0 else ka
                    nc.scalar.activation(tgt[2 * pair][0:64, sl], ps[0:64, :],
                                         AF.Copy, scale=1.0)
                    nc.scalar.activation(tgt[2 * pair + 1][0:64, sl],
                                         ps[64:128, :], AF.Copy, scale=1.0)

            def head_scores(h, s):
                av = psAcc.tile([65, 512], F32, tag="av", name=f"av{l}_{h}_{s}")
                tks = _causal_tk(s)
                for i, tk in enumerate(tks):
                    col0 = max(0, tk * 128 - s * 512)
                    masked = tk * 128 + 127 > s * 512
                    n = 512 - col0
                    sc = psA.tile([128, 512], F32, tag="ps",
                                  name=f"sc{l}_{h}_{s}_{tk}")
                    nc.tensor.matmul(sc[:, :n],
                                     ka[h][:, tk * 128:(tk + 1) * 128],
                                     qa[h][:, s * 512 + col0:(s + 1) * 512],
                                     start=True, stop=True)
                    w4 = spool.tile([128, 512], FP16, tag="w4")
                    sigpipe(spool, sc, w4, col0, masked,
                            on_act=(i % 8 in (0, 3, 6)))
                    nc.tensor.matmul(av[:, col0:],
                                     vaug[:, tk, h * 65:(h + 1) * 65],
                                     w4[:, :n], start=(i == 0),
                                     stop=(i == len(tks) - 1),
                                     skip_group_check=True)
                dd = spool.tile([1, 512], F32, tag="row")
                nc.vector.tensor_scalar(dd[:], av[64:65, :],
                                        scalar1=16.0 * EPS, scalar2=None,
                                        op0=ALU.add, op1=ALU.bypass)
                dr = spool.tile([1, 512], F32, tag="row")
                nc.vector.reciprocal_approx_fast(dr[:], dd[:])
                drb = spool.tile([1, 512], FP16, tag="rowb")
                nc.vector.tensor_copy(drb[:], dr[:])
                rep = psSm.tile([64, 512], F32, tag="rep64",
                                name=f"rep{l}_{h}_{s}")
                nc.tensor.matmul(rep[:], ones_rowb[:, 0:64], drb[:],
                                 start=True, stop=True)
                reps = spool.tile([64, 512], FP16, tag="repsb")
                nc.scalar.activation(reps[:], rep[:], AF.Copy, scale=1.0)
                pair, half = h // 2, h % 2
                nc.vector.tensor_tensor(
                    asb[64 * half:64 * (half + 1), pair,
                        s * 512:(s + 1) * 512],
                    av[0:64, :], reps[:], ALU.mult)

            def outproj(s, idx):
                sl = slice(s * 512, (s + 1) * 512)
                for ot in range(DT):
                    ps = psA.tile([128, 512], F32, tag="ps")
                    for p in range(2):
                        nc.tensor.matmul(ps[:],
                                         wo[:, p, ot * 128:(ot + 1) * 128],
                                         asb[:, p, sl], start=(p == 0),
                                         stop=(p == 1))
                    dl1 = dlpool.tile([128, 512], FP16, tag="dl1",
                                      name=f"dla{l}_{s}_{ot}")
                    nc.scalar.activation(dl1[:], ps[:], AF.Copy, scale=1.0)
                    nc.sync.dma_start(cc_in[idx][ot * 128:(ot + 1) * 128, :],
                                      dl1[:])
                nc.gpsimd.collective_compute(
                    "AllReduce", ALU.add, ins=[cc_in[idx][:]],
                    outs=[cc_out[idx][:]], replica_groups=RG)

            idx0 = (2 * l) * NSTRIP
            # strip 0
            vproj(0)
            qkproj(0)
            for h in range(HPC):
                head_scores(h, 0)
            outproj(0, idx0)
            # deferred addnorm from previous sublayer (strip 1)
            for fn in pending:
                fn()
            # strip 1
            vproj(1)
            qkproj(1)
            for h in range(HPC):
                head_scores(h, 1)
            outproj(1, idx0 + 1)

        def swiglu(l, wpool, wspool, apool, dpool, spool, pending):
            w3sb = wpool.tile([128, NFT_FF, D], FP16, tag="w3")
            nc.scalar.dma_start(w3sb[:],
                              w3T[l].rearrange("(ft p) f -> p ft f", p=128))

            def strip(s, idx):
                hsb = dpool.tile([128, NFT_FF, 512], FP16, tag="hsb",
                                 name=f"hsb{l}_{s}")
                sl = slice(s * 512, (s + 1) * 512)
                for ft in range(NFT_FF):
                    wmg = wspool.tile([128, DT, 128], FP16, tag="wmg")
                    nc.scalar.dma_start(
                        wmg[:], wmT[l][:, ft * 128:(ft + 1) * 128]
                        .rearrange("(dt p) f -> p dt f", p=128))
                    wmv = wspool.tile([128, DT, 128], FP16, tag="wmv")
                    nc.scalar.dma_start(
                        wmv[:],
                        wmT[l][:, DFF_SH + ft * 128:DFF_SH + (ft + 1) * 128]
                        .rearrange("(dt p) f -> p dt f", p=128))
                    gps = psA.tile([128, 512], F32, tag="ps")
                    vps = psA.tile([128, 512], F32, tag="ps")
                    for dt in range(DT):
                        nc.tensor.matmul(gps[:], wmg[:, dt],
                                         xn[:, dt, sl], start=(dt == 0),
                                         stop=(dt == DT - 1))
                    for dt in range(DT):
                        nc.tensor.matmul(vps[:], wmv[:, dt],
                                         xn[:, dt, sl], start=(dt == 0),
                                         stop=(dt == DT - 1))
                    # h = g*(1+u)*v with u = g/(1+|g|)  (x0.5 folded into w3)
                    r = sigrecip(spool, gps, 512, on_act=True)
                    gb = spool.tile([128, 512], FP16, tag="gb")
                    nc.vector.tensor_copy(gb[:], gps[:])
                    u = spool.tile([128, 512], FP16, tag="uu")
                    nc.vector.tensor_tensor(u[:], gb[:], r[:], ALU.mult)
                    t = spool.tile([128, 512], FP16, tag="tt")
                    nc.vector.tensor_scalar(t[:], u[:], scalar1=1.0,
                                            scalar2=None, op0=ALU.add,
                                            op1=ALU.bypass)
                    m2 = spool.tile([128, 512], FP16, tag="m2")
                    nc.vector.tensor_tensor(m2[:], gb[:], t[:], ALU.mult)
                    nc.vector.tensor_tensor(hsb[:, ft], m2[:], vps[:],
                                            ALU.mult)
                for ot in range(DT):
                    ps = psA.tile([128, 512], F32, tag="ps")
                    for ft in range(NFT_FF):
                        nc.tensor.matmul(ps[:],
                                         w3sb[:, ft, ot * 128:(ot + 1) * 128],
                                         hsb[:, ft], start=(ft == 0),
                                         stop=(ft == NFT_FF - 1))
                    dl1 = dlpool.tile([128, 512], FP16, tag="dl1",
                                      name=f"dlm{l}_{s}_{ot}")
                    nc.scalar.activation(dl1[:], ps[:], AF.Copy, scale=1.0)
                    nc.sync.dma_start(cc_in[idx][ot * 128:(ot + 1) * 128, :],
                                      dl1[:])
                nc.gpsimd.collective_compute(
                    "AllReduce", ALU.add, ins=[cc_in[idx][:]],
                    outs=[cc_out[idx][:]], replica_groups=RG)

            idx0 = (2 * l + 1) * NSTRIP
            strip(0, idx0)
            for fn in pending:
                fn()
            strip(1, idx0 + 1)

        npool = ctx.enter_context(tc.tile_pool(name="npool", bufs=1))
        rowpool = ctx.enter_context(tc.tile_pool(name="rowpool", bufs=2))
        with tc.tile_pool(name="wpool", bufs=1) as wpool, \
             tc.tile_pool(name="wspool", bufs=3) as wspool, \
             tc.tile_pool(name="apool", bufs=1) as apool, \
             tc.tile_pool(name="dpool", bufs=2) as dpool, \
             tc.tile_pool(name="scrpool", bufs=4) as scrpool, \
             tc.tile_pool(name="scrfpool", bufs=3) as scrfpool, \
             tc.tile_pool(name="dlpool", bufs=3) as dlpool, \
             tc.tile_pool(name="ewpool", bufs=2) as ewpool, \
             tc.tile_pool(name="w4pool", bufs=2) as w4pool, \
             tc.tile_pool(name="mpool", bufs=2) as mpool:
            spool = PoolSet(aa=scrpool, rr=scrpool, rb=scrpool, uu=scrpool,
                            w2=scrpool, w2m=scrpool, df=scrfpool, rf=scrfpool,
                            af=scrfpool, gb=ewpool, tt=ewpool,
                            m2=ewpool, w4=w4pool, nabs=npool, repb=mpool,
                            repsb=mpool, row=rowpool, rowb=rowpool)

            def an(s, idx):
                return lambda: addnorm(s, idx, spool, dpool)

            # AR chunk ids per layer: 4l (att s0), 4l+1 (att s1),
            # 4l+2 (ffn s0), 4l+3 (ffn s1). Each addnorm is emitted as late
            # as possible so the collective flies under compute: att-s1
            # addnorm lands between the ffn strips, ffn-s1 addnorm between
            # the next layer's attention strips.
            pend = []
            for l in range(L):
                attention(l, wpool, apool, dpool, spool, pend)
                addnorm(0, 4 * l, spool, dpool)
                swiglu(l, wpool, wspool, apool, dpool, spool,
                       [an(1, 4 * l + 1)])
                addnorm(0, 4 * l + 2, spool, dpool)
                pend = [an(1, 4 * l + 3)]
            final_addnorm = pend

        with tc.tile_pool(name="lmw", bufs=4) as lmw, \
             tc.tile_pool(name="lms", bufs=4) as lms:
            nvs = (VOC_SH + 511) // 512
            wts = {}

            def lm_block(vs, tts):
                vw = min(512, VOC_SH - vs * 512)
                if vs not in wts:
                    wt = lmw.tile([128, DT, 512], FP16, tag="wemb",
                                  name=f"wemb{vs % 4}")
                    nc.sync.dma_start(
                        wt[:, :, :vw], membT[:, vs * 512:vs * 512 + vw]
                        .rearrange("(dt p) f -> p dt f", p=128))
                    wts[vs] = wt
                wt = wts[vs]
                for tt in tts:
                    ps = psA.tile([128, 512], F32, tag="ps")
                    for dt in range(DT):
                        nc.tensor.matmul(ps[:, :vw],
                                         xn[:, dt, tt * 128:(tt + 1) * 128],
                                         wt[:, dt, :vw],
                                         start=(dt == 0), stop=(dt == DT - 1))
                    ls = lms.tile([128, 512], F32, tag="lmsb")
                    if tt % 2 == 0:
                        nc.scalar.activation(ls[:, :vw], ps[:, :vw], AF.Copy,
                                             scale=1.0)
                    else:
                        nc.vector.tensor_copy(ls[:, :vw], ps[:, :vw])
                    nc.sync.dma_start(
                        logits[tt * 128:(tt + 1) * 128,
                               vs * 512:vs * 512 + vw],
                        ls[:, :vw])

            # strip-0 token tiles of the first vocab blocks run while the
            # last ffn AR (strip 1) is in flight; its addnorm lands between.
            for vs in range(3):
                lm_block(vs, range(4))
            for fn in final_addnorm:
                fn()
            for vs in range(3):
                lm_block(vs, range(4, DT))
                wts.pop(vs)
            for vs in range(3, nvs):
                lm_block(vs, range(DT))
                wts.pop(vs)
    nc.compile()
    return nc


def _prep_inputs(input_ids, emb, qkv_w, out_w, n1_w, n2_w, wm_w, w3_w, fn_w):
    ids = np.asarray(input_ids)
    emb = np.asarray(emb, dtype=np.float32)
    x0 = emb[ids]                                   # [B, T, D]
    mag = np.mean(np.abs(x0), axis=-1, keepdims=True)
    xn0 = x0 / (mag + EPS)
    iota = np.arange(T, dtype=np.float32)
    qkv_w = np.asarray(qkv_w, dtype=np.float32)
    out_w = np.asarray(out_w, dtype=np.float32)
    wm_w = np.asarray(wm_w, dtype=np.float32)
    w3_w = np.asarray(w3_w, dtype=np.float32)
    n1_w = np.asarray(n1_w, dtype=np.float32)
    n2_w = np.asarray(n2_w, dtype=np.float32)
    fn_w = np.asarray(fn_w, dtype=np.float32)
    per_core = []
    for c in range(NCORES):
        b, r = c // TP, c % TP
        heads = list(range(HPC * r, HPC * r + HPC))
        qa = np.stack([np.stack([-iota, np.full(T, ALIBI[h], np.float32)])
                       for h in heads]).astype(np.float32)
        ka = np.stack([np.stack([np.full(T, ALIBI[h], np.float32), iota])
                       for h in heads]).astype(np.float32)
        wqk = np.empty((L, D, FSH), np.float32)
        wv = np.empty((L, D, VSH), np.float32)
        wo = np.empty((L, VSH, D), np.float32)
        wm = np.zeros((L, D, 2 * DFF_SH), np.float32)
        w3 = np.zeros((L, DFF_SH, D), np.float32)
        for l in range(L):
            q3 = qkv_w[l].reshape(3, H, DH, D)
            qrows = q3[0, heads].reshape(VSH, D) * SCALE
            krows = q3[1, heads].reshape(VSH, D)
            vrows = q3[2, heads].reshape(VSH, D)
            n1 = n1_w[l][:, None]                   # fold into d-rows of W^T
            wqk[l] = np.concatenate([qrows, krows], 0).T * n1
            wv[l] = vrows.T * n1
            ow = out_w[l].reshape(D, H, DH)[:, heads].reshape(D, VSH)
            wo[l] = ow.T
            n2 = n2_w[l][:, None]
            g0, g1 = DFF_SH * r, min(DFF_SH * (r + 1), DFF)
            ng = g1 - g0
            if ng > 0:
                wm[l, :, :ng] = wm_w[l][g0:g1].T * n2
                wm[l, :, DFF_SH:DFF_SH + ng] = wm_w[l][DFF + g0:DFF + g1].T * n2
                w3[l, :ng] = 0.5 * w3_w[l][:, g0:g1].T
        memb = (emb[VOC_SH * r:VOC_SH * (r + 1)] * fn_w[None, :]).T
        per_core.append(dict(
            x0T=np.ascontiguousarray(x0[b].T),
            xn0T=np.ascontiguousarray(xn0[b].T).astype(np.float16),
            qaug=qa, kaug=ka,
            wqkT=np.ascontiguousarray(wqk).astype(np.float16),
            wvT=np.ascontiguousarray(wv).astype(np.float16),
            woT=np.ascontiguousarray(wo).astype(np.float16),
            wmT=np.ascontiguousarray(wm).astype(np.float16),
            w3T=np.ascontiguousarray(w3).astype(np.float16),
            membT=np.ascontiguousarray(memb).astype(np.float16),
        ))
    return per_core


def kernel(**inputs):
    if "nc" not in _CACHE:
        try:
            _CACHE["nc"] = build_nc(use_divide=True)
        except Exception:
            _CACHE["nc"] = build_nc(use_divide=False)
    nc = _CACHE["nc"]
    per_core = _prep_inputs(**inputs)
    res = run_bass_kernel_spmd(nc, per_core, core_ids=list(range(NCORES)),
                               **_CACHE.get("run_kwargs", {}))
    _CACHE["last_result"] = res
    out = np.empty((B, T, V), np.float32)
    for c in range(NCORES):
        b, r = c // TP, c % TP
        out[b, :, VOC_SH * r:VOC_SH * (r + 1)] = res.results[c]["logits"]
    return out
